# revision 30
# baseline (speedup 1.0000x reference)
"""BERT-base forward on 8 Trainium2 NeuronCores, data-parallel over batch.

Each core runs the full 12-layer model on one batch element (512 tokens).
Activations live in SBUF for the whole forward pass; weights stream from HBM
in bf16 (halves DMA vs f32).  All matmul operands are bf16 (1 cycle/row on
the PE, same as f32r for >=256-wide streams, but bf16 stationaries load ~2x
faster).  The f32 residual stream is kept full precision.

Layouts per core (SBUF tiles are [128 partitions, free]):
  token-major  x/y (f32) and x16/y16 (bf16): [128 tok, 4*768]
  hidden-major xT/QT/KT/yT/attnT (bf16): [128 hid, HC, 512]
  V token-major (bf16) [128 tok, TT, 768]; h1T [128 f, FC, 512] bf16.

LayerNorm fast path (valid when LN affines/biases are off, so every residual
input is exactly zero-mean):
  mean: a 385th column on the n=0 Wo / Wd weight blocks holds the FULL row
        sum of the weight matrix, so psum column 384 of the n=0 block IS
        sum_hid(out) -- one Copy per tile extracts it.
  var:  E[x^2] via ACT Square+accum (Square is in every ACT table set).
  rstd: 1/sqrt(v) via quake-III bit seed + 2 Newton steps, BATCHED across
        all 4 token tiles in [128,4] DVE ops -- one short chain per LN
        instead of four, and no ACT table functions at all.
  Normalized output is written twice: bf16 (feeds PE transposes) on DVE,
  f32 (residual) on GPSIMD (off the critical path).
  Transposes are grouped per token tile (6 chunks into one [128,768] bf16
  psum) so the PE starts transposing tile 0 while tiles 1-3 still normalize.

Attention (per head pair c: heads 2c at partitions 0:64, 2c+1 at 64:128):
  S^T[k,q] = matmul(lhsT=KT[d,k-tile], rhs=QT[d,q]) row-packed pairs
  expS = Exp(S^T/8 + maskbias_k) in bf16 (no max subtraction: |scores/8| < 3)
  denom via matmul(lhsT=ones[128,128]) -> sums broadcast across partitions
  O^T = matmul(lhsT=V[:,head cols], rhs=expS) accumulated over k chunks,
  then normalized by 1/sums (+bv) at eviction.

ACT table sets: only Exp and Gelu are used from switchable sets; preload
dummies position the 2 per-layer loads under matmul-busy phases.

Work that is provably a no-op for the given inputs (zero biases, unit
gammas, zero betas, all-ones mask) is skipped at build time; the general
path stays available and is selected per-input on the host.
"""
import os
import numpy as np
import ml_dtypes
from contextlib import ExitStack

import concourse.bass as bass
import concourse.tile as tile
from concourse import bacc, mybir
from concourse import bass_utils

f32 = mybir.dt.float32
f32r = mybir.dt.float32r
bf16 = mybir.dt.bfloat16
i32 = mybir.dt.int32
AF = mybir.ActivationFunctionType
OP = mybir.AluOpType
AX = mybir.AxisListType

V, H, L, NH, I, P, B, S = 30000, 768, 12, 12, 3072, 512, 8, 512
D = H // NH          # 64
HC = H // 128        # 6 hidden chunks
FC = I // 128        # 24 ffn chunks
TT = S // 128        # 4 token tiles
LN_EPS = 1e-3
QUAKE = 0x5F3759DF

LAST_EXEC_TIME_NS = None
LAST_RESULT = None


def _act_preload(nc, pools, func):
    """Dummy 1-elem activation to trigger the ACT table-set load early."""
    vec = pools["vec"]
    j = vec.tile([128, 1], f32, tag="v", name="act_pre")
    nc.vector.memset(j[:], 1.0)
    nc.scalar.activation(j[:], j[:], func)


def _ln_bcast(nc, pools, g_row, b_row, affine):
    if not affine:
        return None, None
    gb = pools["gb"]
    g_bc = gb.tile([128, H], f32, tag="gb", name="g_bc")
    nc.sync.dma_start(g_bc[:], g_row[None, :].partition_broadcast(128))
    b_bc = gb.tile([128, H], f32, tag="gb", name="b_bc")
    nc.sync.dma_start(b_bc[:], b_row[None, :].partition_broadcast(128))
    return g_bc, b_bc


def _ln_finish_tile(nc, pools, z, tt, mu4, ssq4, out16, g_bc, b_bc,
                    f32_on_gpsimd):
    """Per-tile LN finish: v = ssq/H - mu^2 + eps via 2 DVE micro-ops,
    sd = Sqrt(v) on ACT (table preloaded off the critical path), rstd via
    DVE fast reciprocal, then bf16 (DVE) + f32 (GPSIMD) normalized writes."""
    vec = pools["vec"]
    sl = slice(tt * H, (tt + 1) * H)
    mu = mu4[:, tt:tt + 1]
    ssq = ssq4[:, tt:tt + 1]
    A = vec.tile([128, 1], f32, tag="v", name=f"ln_A{tt}")
    nc.vector.tensor_scalar(out=A[:], in0=mu, scalar1=mu,
                            scalar2=float(-1.0 / (H * H)), op0=OP.mult,
                            op1=OP.mult)
    nc.vector.tensor_scalar(out=A[:], in0=A[:], scalar1=float(LN_EPS),
                            scalar2=None, op0=OP.add)
    sd = vec.tile([128, 1], f32, tag="v", name=f"ln_sd{tt}")
    nc.scalar.activation(sd[:], ssq, AF.Sqrt, bias=A[:], scale=float(1.0 / H))
    rstd = vec.tile([128, 1], f32, tag="v", name=f"ln_rstd{tt}")
    nc.vector.reciprocal_approx_fast(out=rstd[:], in_=sd[:])
    mr = vec.tile([128, 1], f32, tag="v", name=f"ln_mr{tt}")
    nc.vector.tensor_scalar(out=mr[:], in0=mu, scalar1=rstd[:],
                            scalar2=float(-1.0 / H), op0=OP.mult, op1=OP.mult)
    if out16 is not None and g_bc is None:
        nc.vector.tensor_scalar(out=out16[:, sl], in0=z[:, sl],
                                scalar1=rstd[:], scalar2=mr[:],
                                op0=OP.mult, op1=OP.add)
    eng = nc.gpsimd if f32_on_gpsimd else nc.vector
    eng.tensor_scalar(out=z[:, sl], in0=z[:, sl], scalar1=rstd[:],
                      scalar2=mr[:], op0=OP.mult, op1=OP.add)
    if g_bc is not None:
        nc.vector.tensor_tensor(out=z[:, sl], in0=z[:, sl], in1=g_bc[:],
                                op=OP.mult)
    if b_bc is not None:
        nc.vector.tensor_tensor(out=z[:, sl], in0=z[:, sl], in1=b_bc[:],
                                op=OP.add)
    if out16 is not None and g_bc is not None:
        nc.vector.tensor_copy(out16[:, sl], z[:, sl])


def _ln_sq(nc, pools, z, tt, ssq4):
    """Sum(z_tile^2) into ssq4[:, tt] via ACT Square+accum."""
    scratch = pools["scratch"]
    sqa = scratch.tile([128, H], f32, tag="sc", name="ln_sqa")
    nc.scalar.activation(sqa[:], z[:, tt * H:(tt + 1) * H], AF.Square,
                         accum_out=ssq4[:, tt:tt + 1])


def _ln_mean_reduce(nc, pools, z, tt, mu4):
    nc.vector.reduce_sum(out=mu4[:, tt:tt + 1], in_=z[:, tt * H:(tt + 1) * H],
                         axis=AX.X)


def _transpose_tile(nc, pools, src16, tt, dst, ident, on_act):
    """Token tile tt of src16 [128 tok, 4*768] -> dst [128 hid, HC, S]
    columns tt*128:(tt+1)*128, via 6 PE transposes into one bf16 psum."""
    psT = pools["psT"]
    tp = psT.tile([128, HC, 128], bf16, tag="tp", name="tp")
    for c in range(HC):
        nc.tensor.transpose(tp[:, c, :],
                            src16[:, tt * H + c * 128: tt * H + c * 128 + 128],
                            ident[:])
    out_view = dst[:, :, tt * 128:tt * 128 + 128]
    if on_act:
        nc.scalar.activation(out_view, tp[:], AF.Copy)
    else:
        nc.vector.tensor_copy(out_view, tp[:])


def build(n_layers=L, flags=None):
    fl = flags or {}
    qk_bias = fl.get("qk_bias", True)
    v_bias = fl.get("v_bias", True)
    o_bias = fl.get("o_bias", True)
    i_bias = fl.get("i_bias", True)
    d_bias = fl.get("d_bias", True)
    ln1_aff = fl.get("ln1_aff", True)
    ln2_aff = fl.get("ln2_aff", True)
    emb_aff = fl.get("emb_aff", True)
    use_mask = fl.get("use_mask", True)
    use_type = fl.get("use_type", True)
    # matmul-mean trick needs every residual input exactly zero-mean
    zm = not (ln1_aff or ln2_aff or emb_aff or o_bias or d_bias)

    nc = bacc.Bacc("TRN2", target_bir_lowering=False, debug=False, num_devices=8)

    dt_in = lambda n, s, d: nc.dram_tensor(n, s, d, kind="ExternalInput").ap()
    ids_d = dt_in("ids", [S], i32)
    tti_d = dt_in("tti", [S], i32)
    mb_d = dt_in("mb", [S], f32)
    tok_d = dt_in("tok_emb", [V, H], f32)
    pos_d = dt_in("pos_emb", [S, H], f32)
    typ_d = dt_in("type_emb", [2, H], f32)
    eg_d = dt_in("emb_g", [H], f32)
    eb_d = dt_in("emb_b", [H], f32)
    wq_d = dt_in("WqS", [L, HC, 128, HC, 128], bf16)
    wk_d = dt_in("WkS", [L, HC, 128, HC, 128], bf16)
    wv_d = dt_in("WvS", [L, 2, 128, HC, 384], bf16)
    wo_d = dt_in("WoA", [L, 2, 128, HC, 385], bf16)
    wi_d = dt_in("WiS", [L, FC, 128, HC, 128], bf16)
    wd_d = dt_in("WdA", [L, FC // 4, 128, 4, 770], bf16)
    bq_d = dt_in("bq", [L, H], f32)
    bk_d = dt_in("bk", [L, H], f32)
    bv_d = dt_in("bv", [L, H], f32)
    bo_d = dt_in("boA", [L, 770], f32r)
    bi_d = dt_in("bi", [L, I], f32)
    bd_d = dt_in("bdA", [L, 770], f32r)
    g1_d = dt_in("ln1_g", [L, H], f32)
    b1_d = dt_in("ln1_b", [L, H], f32)
    g2_d = dt_in("ln2_g", [L, H], f32)
    b2_d = dt_in("ln2_b", [L, H], f32)
    ones_d = dt_in("ones", [128, 128], bf16)
    onesr_d = dt_in("onesr", [1, 128], f32r)
    ident_d = dt_in("ident", [128, 128], bf16)
    out_d = nc.dram_tensor("out", [S, H], f32, kind="ExternalOutput").ap()

    with tile.TileContext(nc) as tc, ExitStack() as ctx:
        actsF = ctx.enter_context(tc.tile_pool(name="actsF", bufs=3))
        acts16 = ctx.enter_context(tc.tile_pool(name="acts16", bufs=7))
        h1p = ctx.enter_context(tc.tile_pool(name="h1p", bufs=1))
        wbig = ctx.enter_context(tc.tile_pool(name="wbig", bufs=3))
        wsmall = ctx.enter_context(tc.tile_pool(name="wsmall", bufs=6))
        wdp = ctx.enter_context(tc.tile_pool(name="wdp", bufs=3))
        gb = ctx.enter_context(tc.tile_pool(name="gb", bufs=2))
        exps_p = ctx.enter_context(tc.tile_pool(name="exps_p", bufs=14))
        bc_p = ctx.enter_context(tc.tile_pool(name="bc_p", bufs=2))
        scratch = ctx.enter_context(tc.tile_pool(name="scratch", bufs=2))
        vec = ctx.enter_context(tc.tile_pool(name="vec", bufs=28))
        brow_p = ctx.enter_context(tc.tile_pool(name="brow_p", bufs=1))
        const = ctx.enter_context(tc.tile_pool(name="const", bufs=1))
        psA = ctx.enter_context(tc.tile_pool(name="psA", bufs=6, space="PSUM"))
        psT = ctx.enter_context(tc.tile_pool(name="psT", bufs=2, space="PSUM"))
        pools = dict(gb=gb, vec=vec, scratch=scratch, psT=psT)

        # constants
        ones_sb = const.tile([128, 128], bf16, tag="ones", name="ones_sb")
        nc.sync.dma_start(ones_sb[:], ones_d[:])
        onesr_sb = const.tile([1, 128], f32r, tag="onesr", name="onesr_sb")
        nc.sync.dma_start(onesr_sb[:], onesr_d[:])
        ident = const.tile([128, 128], bf16, tag="ident", name="ident")
        nc.sync.dma_start(ident[:], ident_d[:])
        ids_sb = const.tile([128, TT], i32, tag="ids", name="ids_sb")
        nc.sync.dma_start(ids_sb[:], ids_d.rearrange("(t p) -> p t", p=128))
        if use_type:
            tti_sb = const.tile([128, TT], i32, tag="tti", name="tti_sb")
            nc.sync.dma_start(tti_sb[:], tti_d.rearrange("(t p) -> p t", p=128))
        if use_mask:
            mb_sb = const.tile([128, TT], f32, tag="mb", name="mb_sb")
            nc.sync.dma_start(mb_sb[:], mb_d.rearrange("(t p) -> p t", p=128))

        _act_preload(nc, pools, AF.Sqrt)

        # ---- embedding ----
        x = actsF.tile([128, TT * H], f32, tag="af", name="x_emb")
        x16 = acts16.tile([128, TT * H], bf16, tag="a16", name="x16_emb")
        eg_bc, eb_bc = _ln_bcast(nc, pools, eg_d, eb_d, emb_aff)
        for tt in range(TT):
            sl = slice(tt * H, (tt + 1) * H)
            nc.gpsimd.indirect_dma_start(
                out=x[:, sl], out_offset=None, in_=tok_d[:],
                in_offset=bass.IndirectOffsetOnAxis(ap=ids_sb[:, tt:tt + 1], axis=0))
            if use_type:
                tmp_t = gb.tile([128, H], f32, tag="gb", name="emb_tmp")
                nc.gpsimd.indirect_dma_start(
                    out=tmp_t[:], out_offset=None, in_=typ_d[:],
                    in_offset=bass.IndirectOffsetOnAxis(ap=tti_sb[:, tt:tt + 1], axis=0))
                nc.vector.tensor_tensor(out=x[:, sl], in0=x[:, sl], in1=tmp_t[:],
                                        op=OP.add)
            tmp_p = gb.tile([128, H], f32, tag="gb", name="emb_pos")
            nc.sync.dma_start(tmp_p[:], pos_d[tt * 128:(tt + 1) * 128, :])
            nc.vector.tensor_tensor(out=x[:, sl], in0=x[:, sl], in1=tmp_p[:], op=OP.add)
        muE = vec.tile([128, TT], f32, tag="v4", name="muE")
        ssqE = vec.tile([128, TT], f32, tag="v4", name="ssqE")
        for tt in range(TT):
            _ln_mean_reduce(nc, pools, x, tt, muE)
            _ln_sq(nc, pools, x, tt, ssqE)
            _ln_finish_tile(nc, pools, x, tt, muE, ssqE, x16, eg_bc, eb_bc,
                            False)
        _act_preload(nc, pools, AF.Exp)

        # ---- layers ----
        for l in range(n_layers):
            xT = acts16.tile([128, HC, S], bf16, tag="a16", name=f"xT_{l}")
            for tt in range(TT):
                _transpose_tile(nc, pools, x16, tt, xT, ident, on_act=(tt % 2 == 1))

            # Q^T, K^T hidden-major bf16
            QT = acts16.tile([128, HC, S], bf16, tag="a16", name=f"QT_{l}")
            KT = acts16.tile([128, HC, S], bf16, tag="a16", name=f"KT_{l}")
            for j in range(HC):
                for dst, w_d, b_d, on_act in ((QT, wq_d, bq_d, True),
                                              (KT, wk_d, bk_d, False)):
                    wblk = wsmall.tile([128, HC, 128], bf16, tag="ws", name="wqk_blk")
                    nc.sync.dma_start(wblk[:], w_d[l, j])
                    pq = psA.tile([128, S], f32, tag="main", name="pq")
                    for ic in range(HC):
                        nc.tensor.matmul(pq[:], lhsT=wblk[:, ic, :],
                                         rhs=xT[:, ic, :],
                                         start=(ic == 0), stop=(ic == HC - 1))
                    if qk_bias:
                        b_sl = vec.tile([128, 1], f32, tag="v", name="bqk_sl")
                        nc.sync.dma_start(b_sl[:], b_d[l, j * 128:(j + 1) * 128][:, None])
                        nc.scalar.activation(dst[:, j, :], pq[:], AF.Identity,
                                             bias=b_sl[:])
                    elif on_act:
                        nc.scalar.activation(dst[:, j, :], pq[:], AF.Copy)
                    else:
                        nc.vector.tensor_copy(dst[:, j, :], pq[:])

            # V token-major bf16
            Vt = acts16.tile([128, TT, H], bf16, tag="a16", name=f"V_{l}")
            for n in range(2):
                wvblk = wbig.tile([128, HC, 384], bf16, tag="wb", name="wv_blk")
                nc.sync.dma_start(wvblk[:], wv_d[l, n])
                for tt in range(TT):
                    pv = psA.tile([128, 384], f32, tag="main", name="pv")
                    for ic in range(HC):
                        nc.tensor.matmul(
                            pv[:], lhsT=xT[:, ic, tt * 128:tt * 128 + 128],
                            rhs=wvblk[:, ic, :],
                            start=(ic == 0), stop=(ic == HC - 1))
                    nc.vector.tensor_copy(
                        Vt[:, tt, n * 384:(n + 1) * 384], pv[:])

            # attention, software-pipelined by one head: while ACT
            # exponentiates head (c,hh)'s scores, the PE runs the PREVIOUS
            # head's denominator + AV matmuls.  Peak psum: 4 score tiles +
            # prev ssum + prev av = 6 = psA bufs.
            attnT = acts16.tile([128, HC, S], bf16, tag="a16", name=f"attnT_{l}")

            def emit_scores(c, hh):
                r0 = 64 * hh
                es = []
                for kc in range(TT):
                    sp = psA.tile([128, S], f32, tag="main", name="sp")
                    nc.tensor.matmul(
                        sp[:],
                        lhsT=KT[r0:r0 + 64, c, kc * 128:kc * 128 + 128],
                        rhs=QT[r0:r0 + 64, c, :],
                        start=True, stop=True)
                    e = exps_p.tile([128, S], bf16, tag="e", name="e")
                    mbias = mb_sb[:, kc:kc + 1] if use_mask else 0.0
                    nc.scalar.activation(e[:], sp[:], AF.Exp,
                                         bias=mbias, scale=0.125)
                    es.append(e)
                return es

            def drain_head(c, hh, es):
                h = 2 * c + hh
                ssum = psA.tile([128, S], f32, tag="main", name="ssum")
                for kc in range(TT):
                    nc.tensor.matmul(ssum[:], lhsT=ones_sb[:, 0:128],
                                     rhs=es[kc][:],
                                     start=(kc == 0), stop=(kc == TT - 1))
                bct = bc_p.tile([128, S], f32, tag="bc", name="bct")
                nc.vector.reciprocal_approx_fast(out=bct[0:64, :],
                                                 in_=ssum[0:64, :])
                av = psA.tile([64, S], f32, tag="main", name="av")
                for kc in range(TT):
                    nc.tensor.matmul(
                        av[:], lhsT=Vt[:, kc, h * D:(h + 1) * D],
                        rhs=es[kc][:],
                        start=(kc == 0), stop=(kc == TT - 1))
                dst = attnT[64 * hh:64 * hh + 64, c, :]
                nc.vector.tensor_tensor(out=dst, in0=av[:, :],
                                        in1=bct[0:64, :], op=OP.mult)
                if v_bias:
                    bv_sl = vec.tile([64, 1], f32, tag="bv", name="bv_sl")
                    nc.sync.dma_start(bv_sl[:], bv_d[l, h * D:(h + 1) * D][:, None])
                    nc.vector.tensor_scalar(out=dst, in0=dst,
                                            scalar1=bv_sl[:], scalar2=None,
                                            op0=OP.add)

            prev = None
            for c in range(HC):
                for hh in range(2):
                    if prev is not None:
                        drain_head(*prev)
                    es = emit_scores(c, hh)
                    prev = (c, hh, es)
            drain_head(*prev)

            # Wo projection (+bo) + residual -> y, LN1 per token tile
            y = actsF.tile([128, TT * H], f32, tag="af", name=f"y_{l}")
            y16 = acts16.tile([128, TT * H], bf16, tag="a16", name=f"y16_{l}")
            g1_bc, b1_bc = _ln_bcast(nc, pools, g1_d[l], b1_d[l], ln1_aff)
            if o_bias:
                bo_row = brow_p.tile([1, 770], f32r, tag="br", name="bo_row")
                nc.sync.dma_start(bo_row[:], bo_d[l][None, :])
            woblks = []
            for n in range(2):
                wob = wbig.tile([128, HC, 385], bf16, tag="wb", name=f"wo_blk{n}")
                nc.sync.dma_start(wob[:], wo_d[l, n])
                woblks.append(wob)
            mu1 = vec.tile([128, TT], f32, tag="v4", name="mu1")
            ssq1 = vec.tile([128, TT], f32, tag="v4", name="ssq1")
            for tt in range(TT):
                for n in range(2):
                    po = psA.tile([128, 385], f32, tag="main", name="po")
                    if o_bias:
                        nc.tensor.matmul(po[:], lhsT=onesr_sb[0:1, :],
                                         rhs=bo_row[0:1, n * 385:(n + 1) * 385],
                                         start=True, stop=False)
                    for jc in range(HC):
                        nc.tensor.matmul(
                            po[:],
                            lhsT=attnT[:, jc, tt * 128:tt * 128 + 128],
                            rhs=woblks[n][:, jc, :],
                            start=(not o_bias and jc == 0), stop=(jc == HC - 1))
                    sl = slice(tt * H + n * 384, tt * H + n * 384 + 384)
                    nc.vector.tensor_tensor(out=y[:, sl], in0=po[:, 0:384],
                                            in1=x[:, sl], op=OP.add)
                    if zm and n == 0:
                        # col 384 of the n=0 block = full-row-sum = H*mean
                        nc.scalar.activation(mu1[:, tt:tt + 1], po[:, 384:385],
                                             AF.Copy)
                if not zm:
                    _ln_mean_reduce(nc, pools, y, tt, mu1)
                _ln_sq(nc, pools, y, tt, ssq1)
                _ln_finish_tile(nc, pools, y, tt, mu1, ssq1, y16, g1_bc,
                                b1_bc, True)
            yT = acts16.tile([128, HC, S], bf16, tag="a16", name=f"yT_{l}")
            for tt in range(TT):
                _transpose_tile(nc, pools, y16, tt, yT, ident, on_act=(tt % 2 == 1))
            _act_preload(nc, pools, AF.Gelu)

            # FFN up: h1T = gelu(yT @ Wi + bi), hidden-major, bf16
            h1T = h1p.tile([128, FC, S], bf16, tag="h1", name=f"h1T_{l}")
            for fc in range(FC):
                wiblk = wsmall.tile([128, HC, 128], bf16, tag="ws", name="wi_blk")
                nc.sync.dma_start(wiblk[:], wi_d[l, fc])
                ph = psA.tile([128, S], f32, tag="main", name="ph")
                for ic in range(HC):
                    nc.tensor.matmul(ph[:], lhsT=wiblk[:, ic, :],
                                     rhs=yT[:, ic, :],
                                     start=(ic == 0), stop=(ic == HC - 1))
                if i_bias:
                    bi_sl = vec.tile([128, 1], f32, tag="v", name="bi_sl")
                    nc.sync.dma_start(bi_sl[:], bi_d[l, fc * 128:(fc + 1) * 128][:, None])
                    nc.scalar.activation(h1T[:, fc, :], ph[:], AF.Gelu,
                                         bias=bi_sl[:])
                else:
                    nc.scalar.activation(h1T[:, fc, :], ph[:], AF.Gelu)

            _act_preload(nc, pools, AF.Sqrt)

            # FFN down (bf16) + bd + residual -> ffnout; waves of 4 (tt,n)
            # pairs; LN2 stats gathered per wave, one batched chain at the end
            ffnout = actsF.tile([128, TT * H], f32, tag="af", name=f"ffnout_{l}")
            f16 = acts16.tile([128, TT * H], bf16, tag="a16", name=f"f16_{l}")
            g2_bc, b2_bc = _ln_bcast(nc, pools, g2_d[l], b2_d[l], ln2_aff)
            if d_bias:
                bd_row = brow_p.tile([1, 770], f32r, tag="br", name="bd_row")
                nc.sync.dma_start(bd_row[:], bd_d[l][None, :])
            mu2 = vec.tile([128, TT], f32, tag="v4", name="mu2")
            ssq2 = vec.tile([128, TT], f32, tag="v4", name="ssq2")
            for wave in range(2):
                tts = (0, 1) if wave == 0 else (2, 3)
                wave_pairs = [(tt, n) for tt in tts for n in range(2)]
                accs = {}
                for (tt, n) in wave_pairs:
                    acc = psA.tile([128, 385], f32, tag="main", name=f"acc{tt}_{n}")
                    if d_bias:
                        nc.tensor.matmul(acc[:], lhsT=onesr_sb[0:1, :],
                                         rhs=bd_row[0:1, n * 385:(n + 1) * 385],
                                         start=True, stop=False)
                    accs[(tt, n)] = acc
                for fp in range(FC // 4):
                    wdblk = wdp.tile([128, 4, 770], bf16, tag="wd", name="wd_blk")
                    nc.sync.dma_start(wdblk[:], wd_d[l, fp])
                    for two in range(4):
                        fc = 4 * fp + two
                        for (tt, n) in wave_pairs:
                            nc.tensor.matmul(
                                accs[(tt, n)][:],
                                lhsT=h1T[:, fc, tt * 128:tt * 128 + 128],
                                rhs=wdblk[:, two, n * 385:(n + 1) * 385],
                                start=(not d_bias and fc == 0), stop=(fc == FC - 1))
                for tt in tts:
                    for n in range(2):
                        sl = slice(tt * H + n * 384, tt * H + n * 384 + 384)
                        nc.vector.tensor_tensor(out=ffnout[:, sl],
                                                in0=accs[(tt, n)][:, 0:384],
                                                in1=y[:, sl], op=OP.add)
                    if zm:
                        nc.scalar.activation(mu2[:, tt:tt + 1],
                                             accs[(tt, 0)][:, 384:385], AF.Copy)
                    else:
                        _ln_mean_reduce(nc, pools, ffnout, tt, mu2)
                    _ln_sq(nc, pools, ffnout, tt, ssq2)
                    _ln_finish_tile(nc, pools, ffnout, tt, mu2, ssq2,
                                    None if l == n_layers - 1 else f16,
                                    g2_bc, b2_bc, True)
            _act_preload(nc, pools, AF.Exp)
            x = ffnout
            x16 = f16

        for tt in range(TT):
            nc.sync.dma_start(out_d[tt * 128:(tt + 1) * 128, :],
                              x[:, tt * H:(tt + 1) * H])

    nc.compile()
    return nc


def _prep_inputs(inputs, b):
    f = np.float32
    b16 = ml_dtypes.bfloat16
    Wq, Wk, Wv, Wo, Wi = (np.asarray(inputs[k], f) for k in ("Wq", "Wk", "Wv", "Wo", "Wi"))
    WqS = np.ascontiguousarray(
        Wq.reshape(L, HC, 128, HC, 128).transpose(0, 3, 2, 1, 4)).astype(b16)
    WkS = np.ascontiguousarray(
        Wk.reshape(L, HC, 128, HC, 128).transpose(0, 3, 2, 1, 4)).astype(b16)
    WvS = np.ascontiguousarray(
        Wv.reshape(L, HC, 128, 2, 384).transpose(0, 3, 2, 1, 4)).astype(b16)
    WiS = np.ascontiguousarray(
        Wi.reshape(L, HC, 128, FC, 128).transpose(0, 3, 2, 1, 4)).astype(b16)
    # Wo augmented: col 384 of the n=0 block = FULL row sum (of bf16 weights)
    Wo16 = Wo.astype(b16)
    WoA = np.zeros((L, 2, 128, HC, 385), b16)
    WoA[..., 0:384] = np.ascontiguousarray(
        Wo16.reshape(L, HC, 128, 2, 384).transpose(0, 3, 2, 1, 4))
    rs = Wo16.astype(f).sum(axis=2)  # [L, 768] full row sums
    WoA[:, 0, :, :, 384] = rs.reshape(L, HC, 128).transpose(0, 2, 1)
    Wd = np.asarray(inputs["Wd"], f)
    Wd16 = Wd.astype(b16)
    WdA = np.zeros((L, FC // 4, 128, 4, 770), b16)
    blk = Wd16.reshape(L, FC // 4, 4, 128, H).transpose(0, 1, 3, 2, 4)
    for n in range(2):
        WdA[..., n * 385:n * 385 + 384] = blk[..., n * 384:(n + 1) * 384]
    WdA[..., 384] = blk.astype(f).sum(axis=-1)  # full row sum on n=0 block
    mask = np.asarray(inputs["input_mask"], f)
    tti = np.asarray(inputs["token_type_ids"], np.int32)
    flags = dict(
        qk_bias=bool(np.any(np.asarray(inputs["bq"])) or np.any(np.asarray(inputs["bk"]))),
        v_bias=bool(np.any(np.asarray(inputs["bv"]))),
        o_bias=bool(np.any(np.asarray(inputs["bo"]))),
        i_bias=bool(np.any(np.asarray(inputs["bi"]))),
        d_bias=bool(np.any(np.asarray(inputs["bd"]))),
        ln1_aff=bool(np.any(np.asarray(inputs["ln1_g"]) != 1.0) or
                     np.any(np.asarray(inputs["ln1_b"]))),
        ln2_aff=bool(np.any(np.asarray(inputs["ln2_g"]) != 1.0) or
                     np.any(np.asarray(inputs["ln2_b"]))),
        emb_aff=bool(np.any(np.asarray(inputs["emb_ln_g"]) != 1.0) or
                     np.any(np.asarray(inputs["emb_ln_b"]))),
        use_mask=bool(np.any(mask != 1.0)),
        use_type=bool(np.any(tti != 0)),
    )
    pos_eff = np.asarray(inputs["pos_emb"], f)[:S]
    if not flags["use_type"]:
        # uniform type ids: fold type_emb[row0] into the position embedding
        pos_eff = pos_eff + np.asarray(inputs["type_emb"], f)[int(tti.flat[0])][None, :]
    boA = np.zeros((L, 770), f)
    boA[:, 0:384] = np.asarray(inputs["bo"], f)[:, 0:384]
    boA[:, 385:769] = np.asarray(inputs["bo"], f)[:, 384:768]
    bdA = np.zeros((L, 770), f)
    bdA[:, 0:384] = np.asarray(inputs["bd"], f)[:, 0:384]
    bdA[:, 385:769] = np.asarray(inputs["bd"], f)[:, 384:768]
    shared = dict(
        tok_emb=np.asarray(inputs["tok_emb"], f),
        pos_emb=pos_eff,
        type_emb=np.asarray(inputs["type_emb"], f),
        emb_g=np.asarray(inputs["emb_ln_g"], f),
        emb_b=np.asarray(inputs["emb_ln_b"], f),
        WqS=WqS, WkS=WkS, WvS=WvS, WoA=WoA, WiS=WiS, WdA=WdA,
        bq=np.asarray(inputs["bq"], f), bk=np.asarray(inputs["bk"], f),
        bv=np.asarray(inputs["bv"], f), boA=boA,
        bi=np.asarray(inputs["bi"], f), bdA=bdA,
        ln1_g=np.asarray(inputs["ln1_g"], f), ln1_b=np.asarray(inputs["ln1_b"], f),
        ln2_g=np.asarray(inputs["ln2_g"], f), ln2_b=np.asarray(inputs["ln2_b"], f),
        ones=np.ones((128, 128), b16),
        onesr=np.ones((1, 128), f),
        ident=np.eye(128, dtype=f).astype(b16),
    )
    in_maps = []
    ids = np.asarray(inputs["input_ids"], np.int32)
    for c in range(b):
        m = dict(shared)
        m["ids"] = np.ascontiguousarray(ids[c])
        m["tti"] = np.ascontiguousarray(tti[c])
        m["mb"] = np.ascontiguousarray((1.0 - mask[c]) * -10000.0)
        in_maps.append(m)
    return in_maps, flags


def kernel(**inputs):
    global LAST_EXEC_TIME_NS, LAST_RESULT
    n_layers = int(os.environ.get("BERT_LAYERS", L))
    trace = bool(os.environ.get("BERT_TRACE"))
    in_maps, flags = _prep_inputs(inputs, B)
    nc = build(n_layers, flags)
    res = bass_utils.run_bass_kernel_spmd(
        nc, in_maps, core_ids=list(range(B)), trace=trace)
    LAST_EXEC_TIME_NS = res.exec_time_ns
    LAST_RESULT = res
    out = np.stack([res.results[c]["out"] for c in range(B)])
    return out.astype(np.float32)
